# revision 16
# baseline (speedup 1.0000x reference)
"""CRF forward (log partition) kernel for Trainium2, 8 NeuronCores.

Math: the reference scan
    alpha_{t+1}[b,i] = logsumexp_j(trans[i,j] + alpha_t[b,j]) + logit[b,t,i]
is computed in scaled probability space.  With A[i,j] = exp(trans[i,j] - K)
(K a constant picked so per-step mass stays ~1) and p_t = exp(alpha_t - t*K),
    p_{t+1}[b,:] = (A @ p_t[b,:]) * exp(logit[b,t,:])
which is a plain matmul chain on the TensorEngine.  Final:
    norm[b] = log(sum_j p_{lens[b]}[b,j] * exp(trans[stop,j] - K)) + (lens[b]+1)*K

Sharding: data-parallel over batch, 16 sequences per core.  Sequence
lengths are handled with a uniform program: every core runs max(lens)
steps; at t == lens[b]-1 the final stop-dot for column b is captured into
a result row via copy_predicated with a host-built one-hot-in-time mask.

Layout per core: state p is [j_partition(128), jtile(4), b(16)] bf16.
Per step: 16 matmuls (lhsT = A^T tile bf16 [128,128], rhs = p slice
[128,16]) accumulate psum[128, 4it, 16]; one DVE tensor_mul by
exp(logit_t) (ACT-exp'd from logits DMA'd in host-pretransposed
[t, j, b] order); 4 tiny matmuls for the stop-dot; one copy_predicated.
"""

import numpy as np

import concourse.bass as bass
import concourse.tile as tile
from concourse import bacc, mybir
from concourse.bass_utils import run_bass_kernel_spmd

B, S, L = 128, 256, 512
NCORES = 8
BLOC = B // NCORES  # 16
START_J = L - 2  # 510
KSHIFT = float(np.log(512.0) + 1.0)
TCHUNK = 8  # time steps per logits DMA

F32 = mybir.dt.float32
BF16 = mybir.dt.bfloat16
EXP = mybir.ActivationFunctionType.Exp
LOG = mybir.ActivationFunctionType.Ln


def _build_program(tmax: int, repeats: int = 1):
    nc = bacc.Bacc(None, target_bir_lowering=False)

    logitsT = nc.dram_tensor("logitsT", [tmax, L, BLOC], F32, kind="ExternalInput")
    trans = nc.dram_tensor("trans", [L, L], F32, kind="ExternalInput")
    snap = nc.dram_tensor("snap", [1, tmax, BLOC], mybir.dt.int32, kind="ExternalInput")
    cvec = nc.dram_tensor("cvec", [1, BLOC], F32, kind="ExternalInput")
    ident = nc.dram_tensor("ident", [128, 128], F32, kind="ExternalInput")
    pzero = nc.dram_tensor("pzero", [128, 4, BLOC], BF16, kind="ExternalInput")
    norm = nc.dram_tensor("norm", [1, BLOC], F32, kind="ExternalOutput")

    with tile.TileContext(nc) as tc:
        with (
            tc.tile_pool(name="const", bufs=1) as constp,
            tc.tile_pool(name="lg", bufs=2) as lgp,
            tc.tile_pool(name="expl", bufs=3) as explp,
            tc.tile_pool(name="pst", bufs=2) as pstp,
            tc.tile_pool(name="ps", bufs=2, space="PSUM") as psp,
            tc.tile_pool(name="pf", bufs=2, space="PSUM") as pfp,
            tc.tile_pool(name="ptr", bufs=4, space="PSUM") as ptrp,
        ):
            # ---- preamble: expTT[jp, jt, it, i_in] = exp(trans[i, j] - K) bf16
            trans_sb = constp.tile([128, 4, L], F32)
            nc.sync.dma_start(
                out=trans_sb,
                in_=trans[:, :].rearrange("(it ip) j -> ip it j", ip=128),
            )
            ident_sb = constp.tile([128, 128], F32)
            nc.sync.dma_start(out=ident_sb, in_=ident[:, :])

            negk_sb = constp.tile([128, 1], F32)
            nc.vector.memset(negk_sb, -KSHIFT)

            expTT = constp.tile([128, 4, 4, 128], BF16)
            for it in range(4):
                for jt in range(4):
                    ptr = ptrp.tile([128, 128], F32)
                    nc.tensor.transpose(
                        ptr[:, :],
                        trans_sb[:, it, jt * 128 : (jt + 1) * 128],
                        ident_sb[:, :],
                    )
                    nc.scalar.activation(
                        expTT[:, jt, it, :], ptr[:, :], EXP, bias=negk_sb[:, :]
                    )

            snap_sb = constp.tile([1, tmax, BLOC], mybir.dt.int32)
            nc.sync.dma_start(out=snap_sb, in_=snap[:, :, :])
            cvec_sb = constp.tile([1, BLOC], F32)
            nc.sync.dma_start(out=cvec_sb, in_=cvec[:, :])
            res_sb = constp.tile([1, BLOC], F32)
            nc.vector.memset(res_sb, 0.0)

            for _rep in range(repeats):
                # p0 = one-hot at start label (510): jt=3, jp=126 (host-built)
                p_cur = pstp.tile([128, 4, BLOC], BF16)
                nc.sync.dma_start(out=p_cur, in_=pzero[:, :, :])

                lg = None
                for t in range(tmax):
                    tcn, ts = divmod(t, TCHUNK)
                    if ts == 0:
                        nt = min(TCHUNK, tmax - tcn * TCHUNK)
                        lg = lgp.tile([128, nt, 4, BLOC], F32)
                        src = logitsT[
                            tcn * TCHUNK : tcn * TCHUNK + nt, :, :
                        ].rearrange("t (jt jp) b -> jp t jt b", jp=128)
                        nc.sync.dma_start(out=lg, in_=src)

                    expl = explp.tile([128, 4, BLOC], F32)
                    nc.scalar.activation(expl[:, :, :], lg[:, ts, :, :], EXP)

                    ps = psp.tile([128, 4, BLOC], F32)
                    p_new = pstp.tile([128, 4, BLOC], BF16)
                    for it in range(4):
                        for jt in range(4):
                            nc.tensor.matmul(
                                ps[:, it, :],
                                expTT[:, jt, it, :],
                                p_cur[:, jt, :],
                                start=(jt == 0),
                                stop=(jt == 3),
                            )
                    nc.vector.tensor_mul(p_new, ps, expl)

                    pf = pfp.tile([1, BLOC], F32)
                    for jt in range(4):
                        # stop row (i=511) lives at it=3, i_in=127 of expTT
                        nc.tensor.matmul(
                            pf[:, :],
                            expTT[:, jt, 3, 127:128],
                            p_new[:, jt, :],
                            start=(jt == 0),
                            stop=(jt == 3),
                        )
                    nc.vector.copy_predicated(
                        res_sb, snap_sb[:, t, :], pf[:, :]
                    )

                    p_cur = p_new

            lnr = constp.tile([1, BLOC], F32)
            nc.scalar.activation(lnr, res_sb, LOG)
            outp = constp.tile([1, BLOC], F32)
            nc.vector.tensor_add(outp, lnr, cvec_sb)
            nc.sync.dma_start(out=norm[:, :], in_=outp)

    if not nc.is_finalized():
        nc.finalize()
    return nc


def _make_in_maps(logits, lens_np, trans, tmax):
    import ml_dtypes

    ident = np.eye(128, dtype=np.float32)
    pzero = np.zeros((128, 4, BLOC), dtype=ml_dtypes.bfloat16)
    pzero[START_J % 128, START_J // 128, :] = 1.0
    in_maps = []
    for c in range(NCORES):
        sl = slice(c * BLOC, (c + 1) * BLOC)
        lgT = np.ascontiguousarray(
            logits[sl, :tmax, :].transpose(1, 2, 0)
        ).astype(np.float32)
        snap = np.zeros((1, tmax, BLOC), np.int32)
        snap[0, lens_np[sl] - 1, np.arange(BLOC)] = 1.0
        cvec = ((lens_np[sl].astype(np.float64) + 1.0) * KSHIFT).astype(
            np.float32
        )[None, :]
        in_maps.append(
            {
                "logitsT": lgT,
                "trans": trans,
                "snap": snap,
                "cvec": cvec,
                "ident": ident,
                "pzero": pzero,
            }
        )
    return in_maps


PROFILE = False  # set by test harness to collect an NTFF trace
LAST_RESULT = None


def kernel(logits, lens, transitions):
    global LAST_RESULT
    logits = np.asarray(logits, dtype=np.float32)
    lens_np = np.asarray(lens).astype(np.int64)
    trans = np.asarray(transitions, dtype=np.float32)
    assert logits.shape == (B, S, L), logits.shape

    tmax = int(lens_np.max())
    nc = _build_program(tmax)
    in_maps = _make_in_maps(logits, lens_np, trans, tmax)
    res = run_bass_kernel_spmd(
        nc, in_maps, core_ids=list(range(NCORES)), trace=PROFILE
    )
    LAST_RESULT = res
    out = np.concatenate([r["norm"][0] for r in res.results])
    return out.astype(np.float32)
